# revision 5
# baseline (speedup 1.0000x reference)
"""GCN feature extractor on 8 Trainium2 NeuronCores.

Row-parallel sharding over the dense normalized adjacency (see module
comment in build_program for the math). Each core computes the full
XW1+b1 (cheap, replicated), its block's contribution to both
adjacency matmuls, and a locally-pooled [OUT, B] partial; the host sums
the 8 partials (the pooling all-reduce) and transposes.
"""

import numpy as np
import ml_dtypes

import concourse.bass as bass
import concourse.mybir as mybir
import concourse.tile as tile
from concourse.vector_clock import ScopedClock
from concourse.bass_utils import run_bass_kernel_spmd

N, FIN, HID, OUT, B, NCORES = 8192, 128, 256, 128, 64, 8
BLK = N // NCORES  # 1024
P = 128

# Matmul operand dtype. bf16 keeps both the PE and HBM sides at the
# roofline ridge; PSUM accumulation is always fp32.
DT = mybir.dt.bfloat16
NP_DT = ml_dtypes.bfloat16

# This walrus build only accepts a single semaphore wait per
# instruction; Tile attaches as many as the dependence structure needs.
# Excess waits are hoisted onto pure-wait EventSemaphore instructions
# (what wait_ge emits) inserted just before the owning instruction.
_CTRL_MAX_WAITS = 1


def _legalize_waits(nc, max_waits=1):
    def fix_block(blk):
        for sub in getattr(blk, "blocks", None) or []:
            fix_block(sub)
        insts = list(blk.instructions)
        out = []
        changed = False
        for inst in insts:
            si = getattr(inst, "sync_info", None)
            waits = list(si.on_wait) if si is not None else []
            if len(waits) > max_waits:
                changed = True
                inst.sync_info = mybir.SyncInfo(
                    on_wait=waits[-max_waits:], on_update=list(si.on_update)
                )
                for j, w in enumerate(waits[: -max_waits]):
                    out.append(
                        mybir.InstEventSemaphore(
                            name=f"{inst.name}-hw{j}",
                            engine=inst.engine,
                            ins=[],
                            outs=[],
                            sync_info=mybir.SyncInfo(on_wait=[w], on_update=[]),
                        )
                    )
            out.append(inst)
        if changed:
            blk.instructions = out

    for fn in nc.m.functions:
        for blk in fn.blocks:
            fix_block(blk)


class _TileContext(tile.TileContext):
    def _drain_and_barrier(self, tick_clock, wait_clock):
        nc = self.nc
        drain_inst = nc.sync.drain()
        wait_clock.add_sem_waits(
            drain_inst.ins, ScopedClock({None: tick_clock.global_clock})
        )
        si = drain_inst.ins.sync_info
        waits = list(si.on_wait) if si is not None else []
        if len(waits) > _CTRL_MAX_WAITS:
            drain_inst.ins.sync_info = mybir.SyncInfo(
                on_wait=waits[:_CTRL_MAX_WAITS], on_update=list(si.on_update)
            )
            rest = waits[_CTRL_MAX_WAITS:]
            for i in range(0, len(rest), _CTRL_MAX_WAITS):
                extra = nc.sync.drain()
                extra.ins.sync_info = mybir.SyncInfo(
                    on_wait=rest[i : i + _CTRL_MAX_WAITS], on_update=[]
                )
        nc.all_engine_barrier()
        popped = nc._tile_sem_poison_stack.pop()
        assert popped is self._sem_poison
        assert self.sems is not None
        nc.clear_and_free_semaphores(list(self.sems.allocated().values()))
        nc.all_engine_barrier()


def build_program(seg_bounds):
    """One SPMD program, identical on all cores (cores differ only in data).

    Math per core c (blk = its 1024 rows of the symmetric norm_adj A):
      XW1b = X @ W1 + b1                       (full N, replicated)
      H1^T = relu(XW1b^T @ A[:, blk])          ([HID, BLK])
      Z    = H1 @ W2 + b2                      ([BLK, OUT])
      Pt   = Z^T @ A[blk, :]                   ([OUT, N]  partial of H2^T)
      pooled[:, g] = sum_{i in seg g} Pt[:, i] * cntinv[g]
    sum over cores of `pooled` == (segment-mean of A @ (H1 W2 + b2))^T.

    seg_bounds: list of (graph, start, end) for non-empty sorted segments.
    """
    nc = bass.Bass()
    f32 = mybir.dt.float32

    # Inputs (host-pretiled layouts; all DMA reads are contiguous runs).
    acol_d = nc.dram_tensor("acol", [64, P, BLK], DT, kind="ExternalInput")
    arow_d = nc.dram_tensor("arow", [16, 8, P, 512], DT, kind="ExternalInput")
    xt_d = nc.dram_tensor("xt", [P, N], DT, kind="ExternalInput")
    w1_d = nc.dram_tensor("w1", [P, HID], DT, kind="ExternalInput")
    b1r_d = nc.dram_tensor("b1r", [P, HID], f32, kind="ExternalInput")
    w2_d = nc.dram_tensor("w2", [2, P, OUT], DT, kind="ExternalInput")
    b2r_d = nc.dram_tensor("b2r", [P, OUT], f32, kind="ExternalInput")
    cinv_d = nc.dram_tensor("cinv", [P, B], f32, kind="ExternalInput")
    out_d = nc.dram_tensor("outp", [P, B], f32, kind="ExternalOutput")

    KC = N // P  # 64 contraction chunks for the layer-1 matmul

    with _TileContext(nc) as tc:
        with (
            tc.tile_pool(name="const", bufs=1) as cpool,
            tc.tile_pool(name="partial", bufs=1) as ppool,
            tc.tile_pool(name="pooled", bufs=1) as opool,
            tc.tile_pool(name="h1t", bufs=1) as hpool,
            tc.tile_pool(name="z", bufs=1) as zpool,
        ):
            # Constants / resident tensors
            w1_sb = cpool.tile([P, HID], DT)
            nc.sync.dma_start(w1_sb[:], w1_d[:])
            b1r_sb = cpool.tile([P, HID], f32)
            nc.sync.dma_start(b1r_sb[:], b1r_d[:])
            w2_sb = [cpool.tile([P, OUT], DT, tag=f"w2_{k}", name=f"w2_{k}") for k in range(2)]
            for k in range(2):
                nc.sync.dma_start(w2_sb[k][:], w2_d[k])
            b2r_sb = cpool.tile([P, OUT], f32)
            nc.sync.dma_start(b2r_sb[:], b2r_d[:])
            cinv_sb = cpool.tile([P, B], f32)
            nc.sync.dma_start(cinv_sb[:], cinv_d[:])

            partial_sb = ppool.tile([P, N], f32)
            pooled_sb = opool.tile([P, B], f32)
            nc.vector.memset(pooled_sb[:], 0.0)

            h1t_sb = [hpool.tile([P, BLK], DT, tag=f"h1t_{m}", name=f"h1t_{m}") for m in range(2)]
            z_sb = [zpool.tile([P, OUT], DT, tag=f"z_{m}", name=f"z_{m}") for m in range(8)]

            with (
                tc.tile_pool(name="xt", bufs=1) as xpool,
                tc.tile_pool(name="xw1", bufs=KC) as wpool,
                tc.tile_pool(name="acol", bufs=3) as apool,
                tc.tile_pool(name="psum_x", bufs=3, space="PSUM") as pxpool,
                tc.tile_pool(name="psum_h", bufs=1, space="PSUM") as phpool,
            ):
                xt_sb = xpool.tile([P, N], DT)
                nc.sync.dma_start(xt_sb[:], xt_d[:])

                # Phase 1: XW1b chunks (nodes-on-partitions, one per K chunk
                # of layer 1) + Phase 2 interleaved by the scheduler.
                xw1_sb = []
                for m in range(KC):
                    ps = pxpool.tile([P, HID], f32, tag="psx", name=f"psx_{m}")
                    nc.tensor.matmul(
                        ps[:],
                        xt_sb[:, m * P : (m + 1) * P],
                        w1_sb[:],
                        start=True,
                        stop=True,
                    )
                    t = wpool.tile([P, HID], DT, tag="xw1", name=f"xw1_{m}")
                    nc.vector.tensor_tensor(
                        t[:], ps[:], b1r_sb[:], mybir.AluOpType.add
                    )
                    xw1_sb.append(t)

                # Phase 2: H1^T = relu(XW1b^T @ Acol), K = N accumulation.
                psh = [
                    [
                        phpool.tile(
                            [P, 512], f32,
                            tag=f"psh_{mc}_{nn}", name=f"psh_{mc}_{nn}",
                        )
                        for nn in range(2)
                    ]
                    for mc in range(2)
                ]
                for k in range(KC):
                    ac = apool.tile([P, BLK], DT, tag="acol", name=f"acol_{k}")
                    nc.sync.dma_start(ac[:], acol_d[k])
                    for mc in range(2):
                        lhsT = xw1_sb[k][:, mc * P : (mc + 1) * P]
                        for nn in range(2):
                            nc.tensor.matmul(
                                psh[mc][nn][:],
                                lhsT,
                                ac[:, nn * 512 : (nn + 1) * 512],
                                start=(k == 0),
                                stop=(k == KC - 1),
                            )
                for mc in range(2):
                    for nn in range(2):
                        nc.scalar.activation(
                            h1t_sb[mc][:, nn * 512 : (nn + 1) * 512],
                            psh[mc][nn][:],
                            mybir.ActivationFunctionType.Relu,
                        )

            with (
                tc.tile_pool(name="arow", bufs=3) as rpool,
                tc.tile_pool(name="psum_z", bufs=2, space="PSUM") as pzpool,
                tc.tile_pool(name="psum_p", bufs=3, space="PSUM") as pppool,
            ):
                # Phase 3: Z = H1 @ W2 + b2  (nodes-on-partitions chunks)
                for mz in range(8):
                    ps = pzpool.tile([P, OUT], f32, tag="psz", name=f"psz_{mz}")
                    for kz in range(2):
                        nc.tensor.matmul(
                            ps[:],
                            h1t_sb[kz][:, mz * P : (mz + 1) * P],
                            w2_sb[kz][:],
                            start=(kz == 0),
                            stop=(kz == 1),
                        )
                    nc.vector.tensor_tensor(
                        z_sb[mz][:], ps[:], b2r_sb[:], mybir.AluOpType.add
                    )

                # Phase 4: partial^T = Z^T @ Arow  ([OUT, N] in fp32)
                for n in range(16):
                    ar = rpool.tile([P, 8, 512], DT, tag="arow", name=f"arow_{n}")
                    nc.sync.dma_start(ar[:], arow_d[n].rearrange("k p f -> p k f"))
                    ps = pppool.tile([P, 512], f32, tag="psp", name=f"psp_{n}")
                    for kz in range(8):
                        nc.tensor.matmul(
                            ps[:],
                            z_sb[kz][:],
                            ar[:, kz, :],
                            start=(kz == 0),
                            stop=(kz == 7),
                        )
                    nc.scalar.activation(
                        partial_sb[:, n * 512 : (n + 1) * 512],
                        ps[:],
                        mybir.ActivationFunctionType.Copy,
                    )

            # Phase 5: segment pooling (sorted batch -> contiguous ranges).
            for g, s, e in seg_bounds:
                nc.vector.reduce_sum(
                    pooled_sb[:, g : g + 1],
                    partial_sb[:, s:e],
                    axis=mybir.AxisListType.X,
                )
            nc.vector.tensor_tensor(
                pooled_sb[:], pooled_sb[:], cinv_sb[:], mybir.AluOpType.mult
            )
            nc.sync.dma_start(out_d[:], pooled_sb[:])

    _legalize_waits(nc)
    return nc


def _host_prep(node_features, W1, b1, W2, b2, edge_index, batch, num_graphs):
    x = np.asarray(node_features, dtype=np.float32)
    W1 = np.asarray(W1, dtype=np.float32)
    b1 = np.asarray(b1, dtype=np.float32)
    W2 = np.asarray(W2, dtype=np.float32)
    b2 = np.asarray(b2, dtype=np.float32)
    ei = np.asarray(edge_index).astype(np.int64)
    batch = np.asarray(batch).astype(np.int64)
    nb = int(num_graphs)

    n = x.shape[0]
    # Dense normalized adjacency, matching the reference exactly:
    # set (dedup) both directions, then add I (so a self-edge gives 2.0).
    A = np.zeros((n, n), dtype=np.float32)
    A[ei[0], ei[1]] = 1.0
    A[ei[1], ei[0]] = 1.0
    A[np.arange(n), np.arange(n)] += 1.0
    deg = A.sum(axis=1, dtype=np.float32)
    dis = np.where(deg > 0, 1.0 / np.sqrt(deg, dtype=np.float32), 0.0).astype(
        np.float32
    )
    A *= dis[:, None]
    A *= dis[None, :]

    counts = np.bincount(batch, minlength=nb).astype(np.int64)
    starts = np.concatenate([[0], np.cumsum(counts)]).astype(np.int64)
    seg_bounds = [
        (g, int(starts[g]), int(starts[g + 1]))
        for g in range(nb)
        if counts[g] > 0
    ]
    cinv = (1.0 / np.maximum(counts, 1)).astype(np.float32)

    xt = np.ascontiguousarray(x.T).astype(NP_DT)  # [FIN, N]
    w1t = W1.astype(NP_DT)  # [FIN, HID]
    b1r = np.broadcast_to(b1, (P, HID)).copy()  # f32
    w2t = W2.reshape(2, P, OUT).astype(NP_DT)
    b2r = np.broadcast_to(b2, (P, OUT)).copy()
    cinvr = np.broadcast_to(cinv, (P, nb)).copy()

    in_maps = []
    for c in range(NCORES):
        lo, hi = c * BLK, (c + 1) * BLK
        acol = np.ascontiguousarray(A[:, lo:hi]).reshape(64, P, BLK).astype(NP_DT)
        arow = (
            np.ascontiguousarray(A[lo:hi, :])
            .reshape(8, P, 16, 512)
            .transpose(2, 0, 1, 3)
            .copy()
            .astype(NP_DT)
        )  # [16 n][8 k][128][512]
        in_maps.append(
            {
                "acol": acol,
                "arow": arow,
                "xt": xt,
                "w1": w1t,
                "b1r": b1r,
                "w2": w2t,
                "b2r": b2r,
                "cinv": cinvr,
            }
        )
    return in_maps, seg_bounds, nb


def kernel(
    node_features, W1, b1, W2, b2, edge_index, batch, num_graphs, **_unused
):
    in_maps, seg_bounds, nb = _host_prep(
        node_features, W1, b1, W2, b2, edge_index, batch, num_graphs
    )
    nc = build_program(seg_bounds)
    res = run_bass_kernel_spmd(nc, in_maps, core_ids=list(range(NCORES)))
    acc = np.zeros((P, B), dtype=np.float32)
    for r in res.results:
        acc += r["outp"]
    return np.ascontiguousarray(acc.T[:nb]).astype(np.float32)


# revision 12
# speedup vs baseline: 1.2361x; 1.2361x over previous
"""GCN feature extractor on 8 Trainium2 NeuronCores.

Row-parallel sharding over the dense normalized adjacency (see module
comment in build_program for the math). Each core computes the full
XW1+b1 (cheap, replicated), its block's contribution to both
adjacency matmuls, and a locally-pooled [OUT, B] partial; the host sums
the 8 partials (the pooling all-reduce) and transposes.
"""

import numpy as np
import ml_dtypes

import concourse.bass as bass
import concourse.mybir as mybir
import concourse.tile as tile
from concourse.vector_clock import ScopedClock
from concourse.bass_utils import run_bass_kernel_spmd

N, FIN, HID, OUT, B, NCORES = 8192, 128, 256, 128, 64, 8
BLK = N // NCORES  # 1024
P = 128

# Matmul operand dtype. bf16 keeps both the PE and HBM sides at the
# roofline ridge; PSUM accumulation is always fp32.
DT = mybir.dt.bfloat16
NP_DT = ml_dtypes.bfloat16

# This walrus build only accepts a single semaphore wait per
# instruction; Tile attaches as many as the dependence structure needs.
# Excess waits are hoisted onto pure-wait EventSemaphore instructions
# (what wait_ge emits) inserted just before the owning instruction.
_CTRL_MAX_WAITS = 1


def _legalize_waits(nc, max_waits=1):
    def fix_block(blk):
        for sub in getattr(blk, "blocks", None) or []:
            fix_block(sub)
        insts = list(blk.instructions)
        out = []
        changed = False
        for inst in insts:
            si = getattr(inst, "sync_info", None)
            waits = list(si.on_wait) if si is not None else []
            if len(waits) > max_waits:
                changed = True
                inst.sync_info = mybir.SyncInfo(
                    on_wait=waits[-max_waits:], on_update=list(si.on_update)
                )
                for j, w in enumerate(waits[: -max_waits]):
                    out.append(
                        mybir.InstEventSemaphore(
                            name=f"{inst.name}-hw{j}",
                            engine=inst.engine,
                            ins=[],
                            outs=[],
                            sync_info=mybir.SyncInfo(on_wait=[w], on_update=[]),
                        )
                    )
            out.append(inst)
        if changed:
            blk.instructions = out

    for fn in nc.m.functions:
        for blk in fn.blocks:
            fix_block(blk)


class _TileContext(tile.TileContext):
    def _drain_and_barrier(self, tick_clock, wait_clock):
        nc = self.nc
        drain_inst = nc.sync.drain()
        wait_clock.add_sem_waits(
            drain_inst.ins, ScopedClock({None: tick_clock.global_clock})
        )
        si = drain_inst.ins.sync_info
        waits = list(si.on_wait) if si is not None else []
        if len(waits) > _CTRL_MAX_WAITS:
            drain_inst.ins.sync_info = mybir.SyncInfo(
                on_wait=waits[:_CTRL_MAX_WAITS], on_update=list(si.on_update)
            )
            rest = waits[_CTRL_MAX_WAITS:]
            for i in range(0, len(rest), _CTRL_MAX_WAITS):
                extra = nc.sync.drain()
                extra.ins.sync_info = mybir.SyncInfo(
                    on_wait=rest[i : i + _CTRL_MAX_WAITS], on_update=[]
                )
        nc.all_engine_barrier()
        popped = nc._tile_sem_poison_stack.pop()
        assert popped is self._sem_poison
        assert self.sems is not None
        nc.clear_and_free_semaphores(list(self.sems.allocated().values()))
        nc.all_engine_barrier()


def build_program(seg_bounds):
    """One SPMD program, identical on all cores (cores differ only in data).

    Math per core c (blk = its 1024 rows of the symmetric norm_adj A):
      XW1b = X @ W1 + b1                       (full N, replicated)
      H1^T = relu(XW1b^T @ A[:, blk])          ([HID, BLK])
      Z    = H1 @ W2 + b2                      ([BLK, OUT])
      Pt   = Z^T @ A[blk, :]                   ([OUT, N]  partial of H2^T)
      pooled[:, g] = sum_{i in seg g} Pt[:, i] * cntinv[g]
    sum over cores of `pooled` == (segment-mean of A @ (H1 W2 + b2))^T.

    seg_bounds: list of (graph, start, end) for non-empty sorted segments.
    """
    nc = bass.Bass()
    f32 = mybir.dt.float32

    # Inputs (host-pretiled layouts; all DMA reads are contiguous runs).
    acol_d = nc.dram_tensor("acol", [64, P, BLK], DT, kind="ExternalInput")
    arow_d = nc.dram_tensor("arow", [16, 8, P, 512], DT, kind="ExternalInput")
    xt_d = nc.dram_tensor("xt", [P, N], DT, kind="ExternalInput")
    w1_d = nc.dram_tensor("w1", [P, HID], DT, kind="ExternalInput")
    b1r_d = nc.dram_tensor("b1r", [P, HID], f32, kind="ExternalInput")
    w2_d = nc.dram_tensor("w2", [2, P, OUT], DT, kind="ExternalInput")
    b2r_d = nc.dram_tensor("b2r", [P, OUT], f32, kind="ExternalInput")
    cinv_d = nc.dram_tensor("cinv", [P, B], f32, kind="ExternalInput")
    out_d = nc.dram_tensor("outp", [P, B], f32, kind="ExternalOutput")

    KC = N // P  # 64 contraction chunks for the layer-1 matmul

    with _TileContext(nc) as tc:
        with (
            tc.tile_pool(name="const", bufs=1) as cpool,
            tc.tile_pool(name="partial", bufs=1) as ppool,
            tc.tile_pool(name="pooled", bufs=1) as opool,
            tc.tile_pool(name="h1t", bufs=1) as hpool,
            tc.tile_pool(name="z", bufs=1) as zpool,
        ):
            # Constants / resident tensors
            w1_sb = cpool.tile([P, HID], DT)
            nc.sync.dma_start(w1_sb[:], w1_d[:])
            b1r_sb = cpool.tile([P, HID], f32)
            nc.sync.dma_start(b1r_sb[:], b1r_d[:])
            w2_sb = [cpool.tile([P, OUT], DT, tag=f"w2_{k}", name=f"w2_{k}") for k in range(2)]
            for k in range(2):
                nc.sync.dma_start(w2_sb[k][:], w2_d[k])
            b2r_sb = cpool.tile([P, OUT], f32)
            nc.sync.dma_start(b2r_sb[:], b2r_d[:])
            cinv_sb = cpool.tile([P, B], f32)
            nc.sync.dma_start(cinv_sb[:], cinv_d[:])

            partial_sb = ppool.tile([P, N], f32)
            pooled_sb = opool.tile([P, B], f32)
            nc.vector.memset(pooled_sb[:], 0.0)

            h1t_sb = [hpool.tile([P, BLK], DT, tag=f"h1t_{m}", name=f"h1t_{m}") for m in range(2)]
            z_sb = [zpool.tile([P, OUT], DT, tag=f"z_{m}", name=f"z_{m}") for m in range(8)]

            # arow loads are interleaved into the phase-2 loop so the
            # second A-block streams while the PE chews on layer 1
            # (otherwise the in-order Sync queue starts them only after
            # all 64 demand-paced acol loads).
            arow_sb = {}

            def load_arow(n):
                t = rpool.tile([P, 8, 512], DT, tag="arow", name=f"arow_{n}")
                nc.sync.dma_start(t[:], arow_d[n].rearrange("k p f -> p k f"))
                arow_sb[n] = t

            rpool_cm = tc.tile_pool(name="arow", bufs=6)
            rpool = rpool_cm.__enter__()
            with (
                tc.tile_pool(name="xt", bufs=1) as xpool,
                tc.tile_pool(name="xw1", bufs=KC) as wpool,
                tc.tile_pool(name="acol", bufs=8) as apool,
                tc.tile_pool(name="psum_x", bufs=3, space="PSUM") as pxpool,
                tc.tile_pool(name="psum_h", bufs=1, space="PSUM") as phpool,
            ):
                xt_sb = xpool.tile([P, N], DT)
                nc.sync.dma_start(xt_sb[:], xt_d[:])

                # Phase 1: XW1b chunks (nodes-on-partitions, one per K chunk
                # of layer 1) + Phase 2 interleaved by the scheduler.
                xw1_sb = []
                for m in range(KC):
                    ps = pxpool.tile([P, HID], f32, tag="psx", name=f"psx_{m}")
                    nc.tensor.matmul(
                        ps[:],
                        xt_sb[:, m * P : (m + 1) * P],
                        w1_sb[:],
                        start=True,
                        stop=True,
                    )
                    t = wpool.tile([P, HID], DT, tag="xw1", name=f"xw1_{m}")
                    nc.vector.tensor_tensor(
                        t[:], ps[:], b1r_sb[:], mybir.AluOpType.add
                    )
                    xw1_sb.append(t)

                # Phase 2: H1^T = relu(XW1b^T @ Acol), K = N accumulation.
                psh = [
                    [
                        phpool.tile(
                            [P, 512], f32,
                            tag=f"psh_{mc}_{nn}", name=f"psh_{mc}_{nn}",
                        )
                        for nn in range(2)
                    ]
                    for mc in range(2)
                ]
                for k in range(KC):
                    ac = apool.tile([P, BLK], DT, tag="acol", name=f"acol_{k}")
                    nc.sync.dma_start(ac[:], acol_d[k])
                    if k >= 40 and k % 4 == 0:
                        load_arow((k - 40) // 4)  # prefetch arow 0..5
                    for mc in range(2):
                        lhsT = xw1_sb[k][:, mc * P : (mc + 1) * P]
                        for nn in range(2):
                            nc.tensor.matmul(
                                psh[mc][nn][:],
                                lhsT,
                                ac[:, nn * 512 : (nn + 1) * 512],
                                start=(k == 0),
                                stop=(k == KC - 1),
                            )
                for mc in range(2):
                    for nn in range(2):
                        nc.scalar.activation(
                            h1t_sb[mc][:, nn * 512 : (nn + 1) * 512],
                            psh[mc][nn][:],
                            mybir.ActivationFunctionType.Relu,
                        )

            with (
                tc.tile_pool(name="psum_z", bufs=2, space="PSUM") as pzpool,
                tc.tile_pool(name="psum_p", bufs=3, space="PSUM") as pppool,
            ):
                # Phase 3: Z = H1 @ W2 + b2  (nodes-on-partitions chunks)
                for mz in range(8):
                    ps = pzpool.tile([P, OUT], f32, tag="psz", name=f"psz_{mz}")
                    for kz in range(2):
                        nc.tensor.matmul(
                            ps[:],
                            h1t_sb[kz][:, mz * P : (mz + 1) * P],
                            w2_sb[kz][:],
                            start=(kz == 0),
                            stop=(kz == 1),
                        )
                    nc.vector.tensor_tensor(
                        z_sb[mz][:], ps[:], b2r_sb[:], mybir.AluOpType.add
                    )

                # Phase 4: partial^T = Z^T @ Arow  ([OUT, N] in fp32)
                for n in range(16):
                    pf = n + 6
                    if 6 <= pf < 16:
                        load_arow(pf)
                    ar = arow_sb[n]
                    ps = pppool.tile([P, 512], f32, tag="psp", name=f"psp_{n}")
                    for kz in range(8):
                        nc.tensor.matmul(
                            ps[:],
                            z_sb[kz][:],
                            ar[:, kz, :],
                            start=(kz == 0),
                            stop=(kz == 7),
                        )
                    nc.scalar.activation(
                        partial_sb[:, n * 512 : (n + 1) * 512],
                        ps[:],
                        mybir.ActivationFunctionType.Copy,
                    )

            rpool_cm.__exit__(None, None, None)

            # Phase 5: segment pooling (sorted batch -> contiguous ranges).
            for g, s, e in seg_bounds:
                nc.vector.reduce_sum(
                    pooled_sb[:, g : g + 1],
                    partial_sb[:, s:e],
                    axis=mybir.AxisListType.X,
                )
            nc.vector.tensor_tensor(
                pooled_sb[:], pooled_sb[:], cinv_sb[:], mybir.AluOpType.mult
            )
            nc.sync.dma_start(out_d[:], pooled_sb[:])

    _legalize_waits(nc)
    return nc


def _host_prep(node_features, W1, b1, W2, b2, edge_index, batch, num_graphs):
    x = np.asarray(node_features, dtype=np.float32)
    W1 = np.asarray(W1, dtype=np.float32)
    b1 = np.asarray(b1, dtype=np.float32)
    W2 = np.asarray(W2, dtype=np.float32)
    b2 = np.asarray(b2, dtype=np.float32)
    ei = np.asarray(edge_index).astype(np.int64)
    batch = np.asarray(batch).astype(np.int64)
    nb = int(num_graphs)

    n = x.shape[0]
    # Dense normalized adjacency, matching the reference exactly:
    # set (dedup) both directions, then add I (so a self-edge gives 2.0).
    A = np.zeros((n, n), dtype=np.float32)
    A[ei[0], ei[1]] = 1.0
    A[ei[1], ei[0]] = 1.0
    A[np.arange(n), np.arange(n)] += 1.0
    deg = A.sum(axis=1, dtype=np.float32)
    dis = np.where(deg > 0, 1.0 / np.sqrt(deg, dtype=np.float32), 0.0).astype(
        np.float32
    )
    A *= dis[:, None]
    A *= dis[None, :]

    counts = np.bincount(batch, minlength=nb).astype(np.int64)
    starts = np.concatenate([[0], np.cumsum(counts)]).astype(np.int64)
    seg_bounds = [
        (g, int(starts[g]), int(starts[g + 1]))
        for g in range(nb)
        if counts[g] > 0
    ]
    cinv = (1.0 / np.maximum(counts, 1)).astype(np.float32)

    xt = np.ascontiguousarray(x.T).astype(NP_DT)  # [FIN, N]
    w1t = W1.astype(NP_DT)  # [FIN, HID]
    b1r = np.broadcast_to(b1, (P, HID)).copy()  # f32
    w2t = W2.reshape(2, P, OUT).astype(NP_DT)
    b2r = np.broadcast_to(b2, (P, OUT)).copy()
    cinvr = np.broadcast_to(cinv, (P, nb)).copy()

    in_maps = []
    for c in range(NCORES):
        lo, hi = c * BLK, (c + 1) * BLK
        acol = np.ascontiguousarray(A[:, lo:hi]).reshape(64, P, BLK).astype(NP_DT)
        arow = (
            np.ascontiguousarray(A[lo:hi, :])
            .reshape(8, P, 16, 512)
            .transpose(2, 0, 1, 3)
            .copy()
            .astype(NP_DT)
        )  # [16 n][8 k][128][512]
        in_maps.append(
            {
                "acol": acol,
                "arow": arow,
                "xt": xt,
                "w1": w1t,
                "b1r": b1r,
                "w2": w2t,
                "b2r": b2r,
                "cinv": cinvr,
            }
        )
    return in_maps, seg_bounds, nb


def kernel(
    node_features, W1, b1, W2, b2, edge_index, batch, num_graphs, **_unused
):
    in_maps, seg_bounds, nb = _host_prep(
        node_features, W1, b1, W2, b2, edge_index, batch, num_graphs
    )
    nc = build_program(seg_bounds)
    res = run_bass_kernel_spmd(nc, in_maps, core_ids=list(range(NCORES)))
    acc = np.zeros((P, B), dtype=np.float32)
    for r in res.results:
        acc += r["outp"]
    return np.ascontiguousarray(acc.T[:nb]).astype(np.float32)


# revision 14
# speedup vs baseline: 1.2693x; 1.0268x over previous
"""GCN feature extractor on 8 Trainium2 NeuronCores.

Row-parallel sharding over the dense normalized adjacency (see module
comment in build_program for the math). Each core computes the full
XW1+b1 (cheap, replicated), its block's contribution to both
adjacency matmuls, and a locally-pooled [OUT, B] partial; the host sums
the 8 partials (the pooling all-reduce) and transposes.
"""

import numpy as np
import ml_dtypes

import concourse.bass as bass
import concourse.mybir as mybir
import concourse.tile as tile
from concourse.vector_clock import ScopedClock
from concourse.bass_utils import run_bass_kernel_spmd

N, FIN, HID, OUT, B, NCORES = 8192, 128, 256, 128, 64, 8
BLK = N // NCORES  # 1024
P = 128

# Matmul operand dtype. bf16 keeps both the PE and HBM sides at the
# roofline ridge; PSUM accumulation is always fp32.
DT = mybir.dt.bfloat16
NP_DT = ml_dtypes.bfloat16

# This walrus build only accepts a single semaphore wait per
# instruction; Tile attaches as many as the dependence structure needs.
# Excess waits are hoisted onto pure-wait EventSemaphore instructions
# (what wait_ge emits) inserted just before the owning instruction.
_CTRL_MAX_WAITS = 1


def _legalize_waits(nc, max_waits=1):
    def fix_block(blk):
        for sub in getattr(blk, "blocks", None) or []:
            fix_block(sub)
        insts = list(blk.instructions)
        out = []
        changed = False
        for inst in insts:
            si = getattr(inst, "sync_info", None)
            waits = list(si.on_wait) if si is not None else []
            if len(waits) > max_waits:
                changed = True
                inst.sync_info = mybir.SyncInfo(
                    on_wait=waits[-max_waits:], on_update=list(si.on_update)
                )
                for j, w in enumerate(waits[: -max_waits]):
                    out.append(
                        mybir.InstEventSemaphore(
                            name=f"{inst.name}-hw{j}",
                            engine=inst.engine,
                            ins=[],
                            outs=[],
                            sync_info=mybir.SyncInfo(on_wait=[w], on_update=[]),
                        )
                    )
            out.append(inst)
        if changed:
            blk.instructions = out

    for fn in nc.m.functions:
        for blk in fn.blocks:
            fix_block(blk)


class _TileContext(tile.TileContext):
    def _drain_and_barrier(self, tick_clock, wait_clock):
        nc = self.nc
        drain_inst = nc.sync.drain()
        wait_clock.add_sem_waits(
            drain_inst.ins, ScopedClock({None: tick_clock.global_clock})
        )
        si = drain_inst.ins.sync_info
        waits = list(si.on_wait) if si is not None else []
        if len(waits) > _CTRL_MAX_WAITS:
            drain_inst.ins.sync_info = mybir.SyncInfo(
                on_wait=waits[:_CTRL_MAX_WAITS], on_update=list(si.on_update)
            )
            rest = waits[_CTRL_MAX_WAITS:]
            for i in range(0, len(rest), _CTRL_MAX_WAITS):
                extra = nc.sync.drain()
                extra.ins.sync_info = mybir.SyncInfo(
                    on_wait=rest[i : i + _CTRL_MAX_WAITS], on_update=[]
                )
        nc.all_engine_barrier()
        popped = nc._tile_sem_poison_stack.pop()
        assert popped is self._sem_poison
        assert self.sems is not None
        nc.clear_and_free_semaphores(list(self.sems.allocated().values()))
        nc.all_engine_barrier()


def build_program(seg_bounds):
    """One SPMD program, identical on all cores (cores differ only in data).

    Math per core c (blk = its 1024 rows of the symmetric norm_adj A):
      XW1b = X @ W1 + b1                       (full N, replicated)
      H1^T = relu(XW1b^T @ A[:, blk])          ([HID, BLK])
      Z    = H1 @ W2 + b2                      ([BLK, OUT])
      Pt   = Z^T @ A[blk, :]                   ([OUT, N]  partial of H2^T)
      pooled[:, g] = sum_{i in seg g} Pt[:, i] * cntinv[g]
    sum over cores of `pooled` == (segment-mean of A @ (H1 W2 + b2))^T.

    seg_bounds: list of (graph, start, end) for non-empty sorted segments.
    """
    nc = bass.Bass()
    f32 = mybir.dt.float32

    # Inputs (host-pretiled layouts; all DMA reads are contiguous runs).
    acol_d = nc.dram_tensor("acol", [64, P, BLK], DT, kind="ExternalInput")
    arow_d = nc.dram_tensor("arow", [16, 8, P, 512], DT, kind="ExternalInput")
    xt_d = nc.dram_tensor("xt", [P, N], DT, kind="ExternalInput")
    w1_d = nc.dram_tensor("w1", [P, HID], DT, kind="ExternalInput")
    b1r_d = nc.dram_tensor("b1r", [P, HID], f32, kind="ExternalInput")
    w2_d = nc.dram_tensor("w2", [2, P, OUT], DT, kind="ExternalInput")
    b2r_d = nc.dram_tensor("b2r", [P, OUT], f32, kind="ExternalInput")
    cinv_d = nc.dram_tensor("cinv", [P, B], f32, kind="ExternalInput")
    out_d = nc.dram_tensor("outp", [P, B], f32, kind="ExternalOutput")

    KC = N // P  # 64 contraction chunks for the layer-1 matmul

    with _TileContext(nc) as tc:
        with (
            tc.tile_pool(name="const", bufs=1) as cpool,
            tc.tile_pool(name="partial", bufs=1) as ppool,
            tc.tile_pool(name="pooled", bufs=1) as opool,
            tc.tile_pool(name="h1t", bufs=1) as hpool,
            tc.tile_pool(name="z", bufs=1) as zpool,
        ):
            # Constants / resident tensors
            w1_sb = cpool.tile([P, HID], DT)
            nc.sync.dma_start(w1_sb[:], w1_d[:])
            b1r_sb = cpool.tile([P, HID], f32)
            nc.sync.dma_start(b1r_sb[:], b1r_d[:])
            w2_sb = [cpool.tile([P, OUT], DT, tag=f"w2_{k}", name=f"w2_{k}") for k in range(2)]
            for k in range(2):
                nc.sync.dma_start(w2_sb[k][:], w2_d[k])
            b2r_sb = cpool.tile([P, OUT], f32)
            nc.sync.dma_start(b2r_sb[:], b2r_d[:])
            cinv_sb = cpool.tile([P, B], f32)
            nc.sync.dma_start(cinv_sb[:], cinv_d[:])

            partial_sb = ppool.tile([P, N], f32)
            pooled_sb = opool.tile([P, B], f32)
            nc.vector.memset(pooled_sb[:], 0.0)

            h1t_sb = [hpool.tile([P, BLK], DT, tag=f"h1t_{m}", name=f"h1t_{m}") for m in range(2)]
            z_sb = [zpool.tile([P, OUT], DT, tag=f"z_{m}", name=f"z_{m}") for m in range(8)]

            # arow loads are interleaved into the phase-2 loop so the
            # second A-block streams while the PE chews on layer 1
            # (otherwise the in-order Sync queue starts them only after
            # all 64 demand-paced acol loads).
            arow_sb = {}

            def load_arow(n):
                t = rpool.tile([P, 8, 512], DT, tag="arow", name=f"arow_{n}")
                nc.sync.dma_start(t[:], arow_d[n].rearrange("k p f -> p k f"))
                arow_sb[n] = t

            rpool_cm = tc.tile_pool(name="arow", bufs=8)
            rpool = rpool_cm.__enter__()
            with (
                tc.tile_pool(name="xt", bufs=1) as xpool,
                tc.tile_pool(name="xw1", bufs=KC) as wpool,
                tc.tile_pool(name="acol", bufs=8) as apool,
                tc.tile_pool(name="psum_x", bufs=3, space="PSUM") as pxpool,
                tc.tile_pool(name="psum_h", bufs=1, space="PSUM") as phpool,
            ):
                # xt in 8 chunk-tiles so the first XW1 matmuls start as
                # soon as the first 512KB lands instead of after the
                # whole 2MB transfer.
                xtc = []
                for j in range(8):
                    t = xpool.tile([P, N // 8], DT, tag=f"xt_{j}", name=f"xt_{j}")
                    nc.sync.dma_start(t[:], xt_d[:, j * (N // 8) : (j + 1) * (N // 8)])
                    xtc.append(t)

                # Phase 1: XW1b chunks (nodes-on-partitions, one per K chunk
                # of layer 1) + Phase 2 interleaved by the scheduler.
                xw1_sb = []
                for m in range(KC):
                    ps = pxpool.tile([P, HID], f32, tag="psx", name=f"psx_{m}")
                    nc.tensor.matmul(
                        ps[:],
                        xtc[m // 8][:, (m % 8) * P : (m % 8 + 1) * P],
                        w1_sb[:],
                        start=True,
                        stop=True,
                    )
                    t = wpool.tile([P, HID], DT, tag="xw1", name=f"xw1_{m}")
                    nc.vector.tensor_tensor(
                        t[:], ps[:], b1r_sb[:], mybir.AluOpType.add
                    )
                    xw1_sb.append(t)

                # Phase 2: H1^T = relu(XW1b^T @ Acol), K = N accumulation.
                psh = [
                    [
                        phpool.tile(
                            [P, 512], f32,
                            tag=f"psh_{mc}_{nn}", name=f"psh_{mc}_{nn}",
                        )
                        for nn in range(2)
                    ]
                    for mc in range(2)
                ]
                for k in range(KC):
                    ac = apool.tile([P, BLK], DT, tag="acol", name=f"acol_{k}")
                    nc.sync.dma_start(ac[:], acol_d[k])
                    if k >= 32 and k % 4 == 0:
                        load_arow((k - 32) // 4)  # prefetch arow 0..7
                    for mc in range(2):
                        lhsT = xw1_sb[k][:, mc * P : (mc + 1) * P]
                        for nn in range(2):
                            nc.tensor.matmul(
                                psh[mc][nn][:],
                                lhsT,
                                ac[:, nn * 512 : (nn + 1) * 512],
                                start=(k == 0),
                                stop=(k == KC - 1),
                            )
                for mc in range(2):
                    for nn in range(2):
                        nc.scalar.activation(
                            h1t_sb[mc][:, nn * 512 : (nn + 1) * 512],
                            psh[mc][nn][:],
                            mybir.ActivationFunctionType.Relu,
                        )

            with (
                tc.tile_pool(name="psum_z", bufs=2, space="PSUM") as pzpool,
                tc.tile_pool(name="psum_p", bufs=3, space="PSUM") as pppool,
            ):
                # Phase 3: Z = H1 @ W2 + b2  (nodes-on-partitions chunks)
                for mz in range(8):
                    ps = pzpool.tile([P, OUT], f32, tag="psz", name=f"psz_{mz}")
                    for kz in range(2):
                        nc.tensor.matmul(
                            ps[:],
                            h1t_sb[kz][:, mz * P : (mz + 1) * P],
                            w2_sb[kz][:],
                            start=(kz == 0),
                            stop=(kz == 1),
                        )
                    nc.vector.tensor_tensor(
                        z_sb[mz][:], ps[:], b2r_sb[:], mybir.AluOpType.add
                    )

                # Phase 4: partial^T = Z^T @ Arow  ([OUT, N] in fp32)
                for n in range(16):
                    pf = n + 8
                    if 8 <= pf < 16:
                        load_arow(pf)
                    ar = arow_sb[n]
                    ps = pppool.tile([P, 512], f32, tag="psp", name=f"psp_{n}")
                    for kz in range(8):
                        nc.tensor.matmul(
                            ps[:],
                            z_sb[kz][:],
                            ar[:, kz, :],
                            start=(kz == 0),
                            stop=(kz == 7),
                        )
                    nc.scalar.activation(
                        partial_sb[:, n * 512 : (n + 1) * 512],
                        ps[:],
                        mybir.ActivationFunctionType.Copy,
                    )

            rpool_cm.__exit__(None, None, None)

            # Phase 5: segment pooling (sorted batch -> contiguous ranges).
            for g, s, e in seg_bounds:
                nc.vector.reduce_sum(
                    pooled_sb[:, g : g + 1],
                    partial_sb[:, s:e],
                    axis=mybir.AxisListType.X,
                )
            nc.vector.tensor_tensor(
                pooled_sb[:], pooled_sb[:], cinv_sb[:], mybir.AluOpType.mult
            )
            nc.sync.dma_start(out_d[:], pooled_sb[:])

    _legalize_waits(nc)
    return nc


def _host_prep(node_features, W1, b1, W2, b2, edge_index, batch, num_graphs):
    x = np.asarray(node_features, dtype=np.float32)
    W1 = np.asarray(W1, dtype=np.float32)
    b1 = np.asarray(b1, dtype=np.float32)
    W2 = np.asarray(W2, dtype=np.float32)
    b2 = np.asarray(b2, dtype=np.float32)
    ei = np.asarray(edge_index).astype(np.int64)
    batch = np.asarray(batch).astype(np.int64)
    nb = int(num_graphs)

    n = x.shape[0]
    # Dense normalized adjacency, matching the reference exactly:
    # set (dedup) both directions, then add I (so a self-edge gives 2.0).
    A = np.zeros((n, n), dtype=np.float32)
    A[ei[0], ei[1]] = 1.0
    A[ei[1], ei[0]] = 1.0
    A[np.arange(n), np.arange(n)] += 1.0
    deg = A.sum(axis=1, dtype=np.float32)
    dis = np.where(deg > 0, 1.0 / np.sqrt(deg, dtype=np.float32), 0.0).astype(
        np.float32
    )
    A *= dis[:, None]
    A *= dis[None, :]

    counts = np.bincount(batch, minlength=nb).astype(np.int64)
    starts = np.concatenate([[0], np.cumsum(counts)]).astype(np.int64)
    seg_bounds = [
        (g, int(starts[g]), int(starts[g + 1]))
        for g in range(nb)
        if counts[g] > 0
    ]
    cinv = (1.0 / np.maximum(counts, 1)).astype(np.float32)

    xt = np.ascontiguousarray(x.T).astype(NP_DT)  # [FIN, N]
    w1t = W1.astype(NP_DT)  # [FIN, HID]
    b1r = np.broadcast_to(b1, (P, HID)).copy()  # f32
    w2t = W2.reshape(2, P, OUT).astype(NP_DT)
    b2r = np.broadcast_to(b2, (P, OUT)).copy()
    cinvr = np.broadcast_to(cinv, (P, nb)).copy()

    in_maps = []
    for c in range(NCORES):
        lo, hi = c * BLK, (c + 1) * BLK
        acol = np.ascontiguousarray(A[:, lo:hi]).reshape(64, P, BLK).astype(NP_DT)
        arow = (
            np.ascontiguousarray(A[lo:hi, :])
            .reshape(8, P, 16, 512)
            .transpose(2, 0, 1, 3)
            .copy()
            .astype(NP_DT)
        )  # [16 n][8 k][128][512]
        in_maps.append(
            {
                "acol": acol,
                "arow": arow,
                "xt": xt,
                "w1": w1t,
                "b1r": b1r,
                "w2": w2t,
                "b2r": b2r,
                "cinv": cinvr,
            }
        )
    return in_maps, seg_bounds, nb


def kernel(
    node_features, W1, b1, W2, b2, edge_index, batch, num_graphs, **_unused
):
    in_maps, seg_bounds, nb = _host_prep(
        node_features, W1, b1, W2, b2, edge_index, batch, num_graphs
    )
    nc = build_program(seg_bounds)
    res = run_bass_kernel_spmd(nc, in_maps, core_ids=list(range(NCORES)))
    acc = np.zeros((P, B), dtype=np.float32)
    for r in res.results:
        acc += r["outp"]
    return np.ascontiguousarray(acc.T[:nb]).astype(np.float32)
